# revision 1
# baseline (speedup 1.0000x reference)
"""Bass/Tile TRN2 kernel for nn_Attention_48653389529729.

reference (jax):
    cat = concat([broadcast(hidden, (S,B,H)), encoder_output], axis=2)  # [S,B,2H]
    energy = tanh(einsum("sbi,hi->sbh", cat, W_attn) + b_attn)          # [S,B,H]
    scores = einsum("sbh,h->sb", energy, v)                             # [S,B]
    out = softmax(scores.T, axis=1)[:, None, :]                        # [B,1,S]

Decomposition: W_attn = [Wh | We] (columns 0:H apply to hidden, H:2H to enc).
    a[b,h]   = hidden[b] @ Wh.T + b_attn   (tiny; precomputed on host)
    E[h,s|b] = We @ enc[:,b,:].T           (the big matmul, fp16 in / fp32 acc)
    scores[b,s] = v . tanh(E + a[b])       (tanh on ACT, v-dot on PE)

Sharding: data-parallel on B across 8 cores (32 b per core); We/v replicated.

Host-side prep (layout only): enc is shipped PRE-TRANSPOSED per core as
encT[p, k, b, s] = enc[s, b, 128k+p], fp16, i zero-padded 500->512 — this
removes all on-device PE transpose-mode ops and the PSUM->SBUF copies that
dominated the previous version (PE array busy dropped ~55us).  We is shipped
pre-transposed [i, h], zero-padded, fp16; a+b_attn as [128, 4, 32] f32 per
core; v as [128, 4, 128] f32r (replicated for the col-group rule).

Device: per batch, 16 fp16 matmuls (4 h-chunks x 4 k-chunks, N=512) accumulate
energyT [h(part), s(free)] in PSUM; ACT applies tanh(+a bias) into f32r SBUF;
4 fp32r matmuls against v reduce over h into scores (fp32r at N=512 streams
1 row/cycle, same as fp16 — no dtype downside).  The v-dot of batch b is
emitted after batch b+1's main matmuls so the PE never waits on ACT.
Per-batch softmax groups of 16 overlap the epilogue with the main loop.
"""

import sys

sys.path.insert(0, "/opt/trn_rl_repo")

import numpy as np

import concourse.mybir as mybir
import concourse.tile as tile
from concourse import bacc
from concourse.bass_utils import run_bass_kernel_spmd

F32 = mybir.dt.float32
F16 = mybir.dt.float16
F32R = mybir.dt.float32r
TANH = mybir.ActivationFunctionType.Tanh
EXP = mybir.ActivationFunctionType.Exp

S, B, H = 512, 256, 500
NCORES = 8
BL = B // NCORES  # 32 batches per core
KC = 128          # i (contraction) chunk size, zero-padded 500 -> 512
NKC = 4           # number of chunks
HP = NKC * KC     # padded i size (512)

_CACHE = {}


def _build(enc_bufs=4, gsz=2, th_bufs=8, psumE_bufs=4, psumS_bufs=2, grp=8,
           n_warm=12):
    nc = bacc.Bacc("TRN2", target_bir_lowering=False)

    encT_d = nc.dram_tensor("encT", [KC, NKC, BL, S], F16, kind="ExternalInput")
    weT_d = nc.dram_tensor("weT", [HP, HP], F16, kind="ExternalInput")
    ab_d = nc.dram_tensor("ab", [KC, NKC, BL], F32, kind="ExternalInput")
    v_d = nc.dram_tensor("v16", [KC, NKC, 128], F32R, kind="ExternalInput")
    out_d = nc.dram_tensor("out", [BL, 1, S], F32, kind="ExternalOutput")

    # chunk schedule: first two batches load individually so the PE can
    # start early; the rest in gsz-sized chunks
    sched = [(0, 1), (1, 1)] + [(b, gsz) for b in range(2, BL, gsz)]
    b2c = {}
    for ci, (b0, n) in enumerate(sched):
        for o in range(n):
            b2c[b0 + o] = (ci, o)

    with tile.TileContext(nc) as tc:
        with (
            tc.tile_pool(name="singles", bufs=1) as singles,
            tc.tile_pool(name="encp1", bufs=2) as encp1,
            tc.tile_pool(name="encp", bufs=enc_bufs) as encp,
        ):
            def load_chunk(ci):
                b0, n = sched[ci]
                pool = encp1 if n == 1 else encp
                t = pool.tile([KC, NKC, n, S], F16, tag=f"enc{n}")
                nc.gpsimd.dma_start(out=t, in_=encT_d[:, :, b0 : b0 + n, :])
                return t

            # weT[p, k, h] = We.T[128k + p, h]; 4 separate tiles so the first
            # matmul only waits on its own k-slice's DMA
            chunks = {0: load_chunk(0)}
            weT = []
            for k in range(NKC):
                wt = singles.tile([KC, HP], F16, tag=f"weT{k}")
                nc.gpsimd.dma_start(out=wt, in_=weT_d[KC * k : KC * (k + 1), :])
                weT.append(wt)
            for ci in (1, 2, 3):
                chunks[ci] = load_chunk(ci)

            ab = singles.tile([KC, NKC, BL], F32)
            nc.sync.dma_start(out=ab, in_=ab_d[:, :, :])
            # v replicated across 128 columns (f32r vdot needs col_grp 0xf)
            v_rep = singles.tile([KC, NKC, 128], F32R)
            nc.sync.dma_start(out=v_rep, in_=v_d[:, :, :])
            # preload the Exp activation table before the tail needs it
            exp_warm = singles.tile([1, 1], F32)
            nc.vector.memset(exp_warm, 0.0)
            nc.scalar.activation(
                out=exp_warm, in_=exp_warm, func=EXP, scale=1.0
            )
            # PE p-state warmup: dummy matmuls while the first DMAs land so
            # the clock is ramped when real work starts
            warm = singles.tile([KC, S], F16, tag="warm")
            nc.vector.memset(warm, 0.0)

            # ---- main loop over local batches ----
            with (
                tc.tile_pool(name="thp", bufs=th_bufs) as thp,
                tc.tile_pool(name="sm", bufs=2) as sm,
                tc.tile_pool(name="ps_E", bufs=psumE_bufs, space="PSUM") as ps_E,
                tc.tile_pool(name="ps_S", bufs=psumS_bufs, space="PSUM") as ps_S,
                tc.tile_pool(name="ps_W", bufs=1, space="PSUM") as ps_W,
            ):
                psW = ps_W.tile([KC, S], F32, tag="psW")
                for _ in range(n_warm):
                    nc.tensor.matmul(
                        psW, warm[:, 0:KC], warm, start=True, stop=True
                    )

                GRP = grp  # softmax group size
                sc_group = None
                pend = None  # deferred v-dot: (ths, bi)

                def do_vdot(ths, bi):
                    nonlocal sc_group
                    psS = ps_S.tile([KC, S], F32, tag="psS")
                    for m in range(NKC):
                        nc.tensor.matmul(
                            psS,
                            v_rep[:, m, :],
                            ths[m],
                            start=(m == 0),
                            stop=(m == NKC - 1),
                        )
                    if bi % GRP == 0:
                        sc_group = sm.tile([GRP, S], F32, tag="scg")
                    strip = sm.tile([1, S], F32, tag="strip")
                    nc.vector.tensor_copy(strip, psS[0:1, :])
                    nc.gpsimd.dma_start(
                        out=sc_group[bi % GRP : bi % GRP + 1, :], in_=strip
                    )
                    if bi % GRP == GRP - 1:
                        # softmax for this group of GRP batches
                        g = bi - GRP + 1
                        negmax = sm.tile([GRP, 1], F32, tag="negmax")
                        nc.vector.reduce_max(
                            negmax,
                            sc_group,
                            axis=mybir.AxisListType.X,
                            negate=True,
                        )
                        probs = sm.tile([GRP, S], F32, tag="probs")
                        sums = sm.tile([GRP, 1], F32, tag="sums")
                        nc.scalar.activation(
                            out=probs,
                            in_=sc_group,
                            func=EXP,
                            bias=negmax,
                            scale=1.0,
                            accum_out=sums,
                        )
                        rinv = sm.tile([GRP, 1], F32, tag="rinv")
                        nc.vector.reciprocal(rinv, sums)
                        nc.vector.tensor_scalar_mul(probs, probs, rinv)
                        nc.sync.dma_start(
                            out=out_d[g : bi + 1, :, :],
                            in_=probs.rearrange("b (one s) -> b one s", one=1),
                        )

                NCHUNK = len(sched)
                for bi in range(BL):
                    ci, off = b2c[bi]
                    et = chunks[ci]
                    if off == 0 and ci + 4 < NCHUNK and ci + 4 not in chunks:
                        chunks[ci + 4] = load_chunk(ci + 4)
                    ths = []
                    for m in range(NKC):
                        psE = ps_E.tile([KC, S], F32, tag="psE")
                        for k in range(NKC):
                            nc.tensor.matmul(
                                psE,
                                weT[k][:, KC * m : KC * (m + 1)],
                                et[:, k, off, :],
                                start=(k == 0),
                                stop=(k == NKC - 1),
                            )
                        th = thp.tile([KC, S], F32R, tag="th")
                        nc.scalar.activation(
                            out=th,
                            in_=psE,
                            func=TANH,
                            bias=ab[:, m, bi : bi + 1],
                            scale=1.0,
                        )
                        ths.append(th)
                    # v-dot of the PREVIOUS batch lands behind this batch's
                    # matmuls in the PE queue, so its tanh inputs are long
                    # ready and the PE never stalls on ACT.
                    if pend is not None:
                        do_vdot(*pend)
                    pend = (ths, bi)
                    if off == sched[ci][1] - 1:
                        chunks.pop(ci, None)
                do_vdot(*pend)

    nc.compile()
    return nc


def _get_nc(**kw):
    key = tuple(sorted(kw.items()))
    if key not in _CACHE:
        _CACHE[key] = _build(**kw)
    return _CACHE[key]


def kernel(hidden, encoder_output, W_attn, b_attn, v, **run_kw):
    hidden = np.asarray(hidden, dtype=np.float32)
    encoder_output = np.asarray(encoder_output, dtype=np.float32)
    W_attn = np.asarray(W_attn, dtype=np.float32)
    b_attn = np.asarray(b_attn, dtype=np.float32)
    v = np.asarray(v, dtype=np.float32)

    # host-side layout prep (cheap, one-shot)
    enc16 = np.zeros((S, B, HP), dtype=np.float16)
    enc16[:, :, :H] = encoder_output
    # encT[p, k, b, s] = enc[s, b, 128k + p]
    encT = np.ascontiguousarray(
        enc16.transpose(2, 1, 0).reshape(NKC, KC, B, S).transpose(1, 0, 2, 3)
    )
    weT = np.zeros((HP, HP), dtype=np.float16)
    weT[:H, :H] = W_attn[:, H:].T.astype(np.float16)         # [i, h], padded
    a_full = np.zeros((B, HP), dtype=np.float32)
    a_full[:, :H] = hidden[0] @ W_attn[:, :H].T + b_attn     # [B, H] f32
    vpad = np.zeros(HP, dtype=np.float32)
    vpad[:H] = v
    v16 = np.ascontiguousarray(
        np.repeat(vpad.reshape(NKC, KC).T[:, :, None], 128, axis=2)
    ).astype(np.float32)

    nc = _get_nc()
    in_maps = []
    for c in range(NCORES):
        sl = slice(c * BL, (c + 1) * BL)
        ab_core = np.ascontiguousarray(
            a_full[sl].T.reshape(NKC, KC, BL).transpose(1, 0, 2)
        ).astype(np.float32)                                 # [128, 4, 32]
        in_maps.append(
            {
                "encT": np.ascontiguousarray(encT[:, :, sl, :]),
                "weT": weT,
                "ab": ab_core,
                "v16": v16,
            }
        )
    res = run_bass_kernel_spmd(
        nc, in_maps, core_ids=list(range(NCORES)), **run_kw
    )
    out = np.concatenate([res.results[c]["out"] for c in range(NCORES)], axis=0)
    if run_kw:
        return out.astype(np.float32), res
    return out.astype(np.float32)



# revision 12
# speedup vs baseline: 1.2201x; 1.2201x over previous
"""Bass/Tile TRN2 kernel for nn_Attention_48653389529729.

reference (jax):
    cat = concat([broadcast(hidden, (S,B,H)), encoder_output], axis=2)  # [S,B,2H]
    energy = tanh(einsum("sbi,hi->sbh", cat, W_attn) + b_attn)          # [S,B,H]
    scores = einsum("sbh,h->sb", energy, v)                             # [S,B]
    out = softmax(scores.T, axis=1)[:, None, :]                        # [B,1,S]

v2 design ([s,h] layout — no PE v-dot):
    The v1 kernel computed E in [h(part), s(free)] layout, which made the
    v-reduction a partition reduce that only the PE can do: 4 extra fp32r
    matmuls per batch = ~27us of PE time on top of the ~107us of main
    matmuls.  v2 flips the layout: E[s(part), h(free)] via
        matmul(lhsT=encT[i, s-chunk] (stationary), rhs=WeT[i, h] (moving))
    so the v-reduction becomes a FREE-axis reduce: one DVE
    tensor_tensor_reduce (th * v_bcast, accum=add) per [128, 500] chunk.
    PE now runs ONLY the 16 main matmuls per batch (ap=500 -> 208ns each).

    The bias a[b,h] = hidden[b] @ Wh.T + b_attn can't ride along in this
    layout (ACT bias is per-partition = per-s here), so it is folded into
    the encoder input ON THE HOST:  We @ (enc + delta[b]) = We@enc + a
    with delta[b] = pinv_hi(We) @ a[b] over the well-conditioned singular
    modes, and the rank-12 residual (worst 1/sigma modes) shipped through
    the 12 zero-pad contraction rows (i=500..511):  stationary rows get
    alpha[b,j] = u_j.a[b], moving rows get u_j.  Exact in real arithmetic;
    fp16 cost of the delta shift measured 8.0e-3 end-to-end (tol 2e-2).

    scores accumulate as sc_all[p, 32*sc + b] = scores[b, 128*sc + p]; one
    PE transpose + 4 partition-aligned DMAs reassemble V[b, s]; a single
    [32, 512] softmax + one output DMA finish the kernel.
"""

import sys

sys.path.insert(0, "/opt/trn_rl_repo")

import numpy as np

import concourse.mybir as mybir
import concourse.tile as tile
from concourse import bacc
from concourse.bass_utils import run_bass_kernel_spmd

F32 = mybir.dt.float32
F16 = mybir.dt.float16
TANH = mybir.ActivationFunctionType.Tanh
EXP = mybir.ActivationFunctionType.Exp
MULT = mybir.AluOpType.mult
ADD = mybir.AluOpType.add

S, B, H = 512, 256, 500
NCORES = 8
BL = B // NCORES  # 32 batches per core
KC = 128          # i (contraction) chunk size, zero-padded 500 -> 512
NKC = 4           # number of contraction chunks
NSC = 4           # number of s-partition chunks (512 / 128)
HP = NKC * KC     # padded i size (512)
RLOW = HP - H     # 12 low-sigma residual modes through the pad rows

_CACHE = {}


def _build(enc_bufs=4, gsz=2, th_bufs=6, scr_bufs=2, psumE_bufs=6, n_warm=12,
           variant="full"):
    nc = bacc.Bacc("TRN2", target_bir_lowering=False)

    # encT[p, k, b, sc, s] = enc'[i=128k+p, b, 128*sc+s]  (fp16, delta-shifted)
    encT_d = nc.dram_tensor("encT", [KC, NKC, BL, NSC, KC], F16,
                            kind="ExternalInput")
    # weT[p, k, h] = WeP[128k+p, h]  (fp16; rows 500.. are u_j residual rows)
    weT_d = nc.dram_tensor("weT", [KC, NKC, H], F16, kind="ExternalInput")
    # v broadcast across partitions: [128, 500] f32
    v_d = nc.dram_tensor("vb", [KC, H], F32, kind="ExternalInput")
    ident_d = nc.dram_tensor("ident", [KC, KC], F32, kind="ExternalInput")
    out_d = nc.dram_tensor("out", [BL, 1, S], F32, kind="ExternalOutput")

    # chunk schedule: first two batches load individually (k-split for the
    # very first) so the PE can start early; the rest in gsz-sized chunks
    sched = [(0, 1), (1, 1)] + [(b, gsz) for b in range(2, BL, gsz)]
    b2c = {}
    for ci, (b0, n) in enumerate(sched):
        for o in range(n):
            b2c[b0 + o] = (ci, o)

    with tile.TileContext(nc) as tc:
        with (
            tc.tile_pool(name="singles", bufs=1) as singles,
            tc.tile_pool(name="encp1", bufs=2) as encp1,
            tc.tile_pool(name="encp", bufs=enc_bufs) as encp,
        ):
            def load_chunk(ci):
                b0, n = sched[ci]
                pool = encp1 if n == 1 else encp
                t = pool.tile([KC, NKC, n, NSC, KC], F16, tag=f"enc{n}")
                if ci == 0:
                    # k-split so the k=0 slice lands first
                    for k in range(NKC):
                        nc.gpsimd.dma_start(
                            out=t[:, k, :, :, :],
                            in_=encT_d[:, k, b0 : b0 + n, :, :],
                        )
                else:
                    nc.gpsimd.dma_start(
                        out=t, in_=encT_d[:, :, b0 : b0 + n, :, :]
                    )
                return t

            chunks = {0: load_chunk(0)}
            weT = []
            for k in range(NKC):
                wt = singles.tile([KC, H], F16, tag=f"weT{k}")
                nc.sync.dma_start(out=wt, in_=weT_d[:, k, :])
                weT.append(wt)
            for ci in (1, 2, 3):
                chunks[ci] = load_chunk(ci)

            v_bcast = singles.tile([KC, H], F32)
            nc.sync.dma_start(out=v_bcast, in_=v_d[:, :])
            ident = singles.tile([KC, KC], F32)
            nc.sync.dma_start(out=ident, in_=ident_d[:, :])
            # scores accumulator: sc_all[p, 32*sc + b] = scores[b, 128*sc+p]
            sc_all = singles.tile([KC, KC], F32)
            # preload the Exp activation table before the tail needs it
            exp_warm = singles.tile([1, 1], F32)
            nc.vector.memset(exp_warm, 0.0)
            nc.scalar.activation(out=exp_warm, in_=exp_warm, func=EXP, scale=1.0)
            # PE p-state warmup: dummy matmuls while the first DMAs land
            warm = singles.tile([KC, S], F16, tag="warm")
            nc.vector.memset(warm, 0.0)

            with (
                tc.tile_pool(name="thp", bufs=th_bufs) as thp,
                tc.tile_pool(name="scrp", bufs=scr_bufs) as scrp,
                tc.tile_pool(name="sm", bufs=1) as sm,
                tc.tile_pool(name="ps_E", bufs=psumE_bufs, space="PSUM") as ps_E,
                tc.tile_pool(name="ps_W", bufs=1, space="PSUM") as ps_W,
                tc.tile_pool(name="ps_T", bufs=1, space="PSUM") as ps_T,
            ):
                psW = ps_W.tile([KC, S], F32, tag="psW")
                for _ in range(n_warm):
                    nc.tensor.matmul(
                        psW, warm[:, 0:KC], warm, start=True, stop=True
                    )

                NCHUNK = len(sched)
                for bi in range(BL):
                    ci, off = b2c[bi]
                    et = chunks[ci]
                    if off == 0 and ci + 4 < NCHUNK and ci + 4 not in chunks:
                        chunks[ci + 4] = load_chunk(ci + 4)
                    for sc in range(NSC):
                        # full-bank (2048B) PSUM tile; matmul writes [:, :H]
                        psE = ps_E.tile([KC, S], F32, tag="psE")
                        for k in range(NKC):
                            nc.tensor.matmul(
                                psE[:, 0:H],
                                et[:, k, off, sc, :],
                                weT[k],
                                start=(k == 0),
                                stop=(k == NKC - 1),
                            )
                        if variant == "noact":
                            continue
                        th = thp.tile([KC, H], F32, tag="th")
                        nc.scalar.activation(out=th, in_=psE[:, 0:H], func=TANH)
                        if variant == "nodve":
                            continue
                        col = 32 * sc + bi
                        if variant == "nottr":
                            # bisect: plain unfused reduce instead of ttr
                            nc.vector.reduce_sum(
                                sc_all[:, col : col + 1],
                                th,
                                axis=mybir.AxisListType.X,
                            )
                        else:
                            # custom-DVE ucode op: out = (th*1+0)*v,
                            # accum_out = sum over free axis.  (The native
                            # TENSOR_TENSOR_REDUCE ISA opcode crashes the
                            # exec unit on this runtime.)
                            scr = scrp.tile([KC, H], F32, tag="scr")
                            nc.vector.affine_mul_reduce(
                                out=scr,
                                accum_out=sc_all[:, col : col + 1],
                                in0=th,
                                in1=v_bcast,
                                scale=1.0,
                                bias=0.0,
                            )
                    if off == sched[ci][1] - 1:
                        chunks.pop(ci, None)

                # ---- epilogue: transpose, reassemble, one softmax ----
                sc_t = sm.tile([KC, KC], F32, tag="sc_t")
                if variant in ("noact", "nodve"):
                    # bisect: no scores were produced; fabricate some
                    nc.vector.memset(sc_t, 0.125)
                elif variant == "notrans":
                    # bisect: skip the PE transpose (wrong values, same plumbing)
                    nc.vector.tensor_copy(sc_t, sc_all)
                else:
                    # full-bank PSUM tile; transpose writes [:, :KC]
                    psT = ps_T.tile([KC, S], F32, tag="psT")
                    nc.tensor.transpose(psT[:, 0:KC], sc_all, ident)
                    # DMA cannot read PSUM: bounce transposed scores to SBUF
                    nc.vector.tensor_copy(sc_t, psT[:, 0:KC])
                V = sm.tile([BL, S], F32, tag="V")
                for sc in range(NSC):
                    nc.sync.dma_start(
                        out=V[:, KC * sc : KC * (sc + 1)],
                        in_=sc_t[32 * sc : 32 * sc + 32, :],
                    )
                negmax = sm.tile([BL, 1], F32, tag="negmax")
                nc.vector.reduce_max(
                    negmax, V, axis=mybir.AxisListType.X, negate=True
                )
                probs = sm.tile([BL, S], F32, tag="probs")
                sums = sm.tile([BL, 1], F32, tag="sums")
                nc.scalar.activation(
                    out=probs,
                    in_=V,
                    func=EXP,
                    bias=negmax,
                    scale=1.0,
                    accum_out=sums,
                )
                rinv = sm.tile([BL, 1], F32, tag="rinv")
                nc.vector.reciprocal(rinv, sums)
                nc.vector.tensor_scalar_mul(probs, probs, rinv)
                nc.sync.dma_start(
                    out=out_d[:, :, :],
                    in_=probs.rearrange("b (one s) -> b one s", one=1),
                )

    nc.compile()
    return nc


def _get_nc(**kw):
    key = tuple(sorted(kw.items()))
    if key not in _CACHE:
        _CACHE[key] = _build(**kw)
    return _CACHE[key]


def kernel(hidden, encoder_output, W_attn, b_attn, v, **run_kw):
    hidden = np.asarray(hidden, dtype=np.float64)
    encoder_output = np.asarray(encoder_output, dtype=np.float32)
    W_attn = np.asarray(W_attn, dtype=np.float64)
    b_attn = np.asarray(b_attn, dtype=np.float64)
    v = np.asarray(v, dtype=np.float32)

    # ---- host-side bias fold (one-shot, f64) ----
    Wh, We = W_attn[:, :H], W_attn[:, H:]
    a = hidden[0] @ Wh.T + b_attn                       # [B, H]
    U, sig, Vt = np.linalg.svd(We)
    hi = slice(0, H - RLOW)
    # delta[b] = V diag(1/sig_hi) U_hi^T a[b]   -> We @ delta = P_hi a
    delta = Vt[hi].T @ ((U[:, hi].T @ a.T) / sig[hi, None])   # [H, B]
    alpha = U[:, H - RLOW :].T @ a.T                          # [RLOW, B]

    # enc'[i, b, s]: rows 0..499 = enc + delta (bcast over s), rows 500.. = alpha
    encp = np.empty((HP, B, S), dtype=np.float32)
    encp[:H] = encoder_output.transpose(2, 1, 0) + delta[:, :, None].astype(
        np.float32
    )
    encp[H:] = alpha[:, :, None].astype(np.float32)
    # encT[p, k, b, sc, s] = enc'[128k+p, b, 128*sc+s]  fp16
    encT = np.ascontiguousarray(
        encp.reshape(NKC, KC, B, NSC, KC).transpose(1, 0, 2, 3, 4)
    ).astype(np.float16)

    weP = np.empty((HP, H), dtype=np.float64)
    weP[:H] = We.T
    weP[H:] = U[:, H - RLOW :].T
    weT = np.ascontiguousarray(
        weP.reshape(NKC, KC, H).transpose(1, 0, 2)
    ).astype(np.float16)

    v_bcast = np.ascontiguousarray(
        np.broadcast_to(v[None, :], (KC, H))
    ).astype(np.float32)
    ident = np.eye(KC, dtype=np.float32)

    nc = _get_nc()
    in_maps = []
    for c in range(NCORES):
        sl = slice(c * BL, (c + 1) * BL)
        in_maps.append(
            {
                "encT": np.ascontiguousarray(encT[:, :, sl, :, :]),
                "weT": weT,
                "vb": v_bcast,
                "ident": ident,
            }
        )
    res = run_bass_kernel_spmd(
        nc, in_maps, core_ids=list(range(NCORES)), **run_kw
    )
    out = np.concatenate([res.results[c]["out"] for c in range(NCORES)], axis=0)
    if run_kw:
        return out.astype(np.float32), res
    return out.astype(np.float32)
